# revision 18
# baseline (speedup 1.0000x reference)
"""Trainium2 Bass kernel for nn_Aligner (cross-attention aligner).

Math (per batch element i):
    ex      = ix[i] @ W.T + b          # [L, D]
    eother  = iother[i] @ W.T + b      # [L, D]
    align   = softmax(ex @ eother.T)   # [L, L], softmax over last dim
    out[i]  = align @ iother[i]        # [L, D]

Shapes: B=8, L=2048, D=1024, fp32.  Sharding: batch-parallel, one batch
element per NeuronCore (8 cores), W/b replicated.  No collectives.

All matmuls run in float32r (full PE rate at N=512).  TRN2 fp32r rounds
matmul inputs to 11 mantissa bits (RNE, HW-verified); engine writes into
f32r tiles round the same way.  An 11-bit logit pipeline is ~3e-2 off
the fp32 reference, so precision is recovered in two places:

1. hi/lo splits: any value x is stored as xh = rne11(x) (exact on the
   PE) plus xl = rne11(x - xh).  A product X@Y is computed in three
   f32r passes Xh@Yh + Xh@Yl + Xl@Yh (error ~2^-23).  Used for the
   stage-1 projections AND the align matmul (ex/eother are stored as
   hi/lo pairs).
2. chunk-relative logits: align PSUM chunks are stored as
   S_c - max(S_c) so the entries that survive softmax have tiny
   absolute rounding error; exp applies the per-chunk offset
   (max_c - rowmax) via its per-partition bias, with accum_out giving
   per-chunk row sums.  1/Z is folded into the output eviction.

Measured end-to-end max-scale-relative error of this scheme: ~1.8e-4.

Per-core dataflow:
  phase A: W -> WTh/WTl (split + PE transpose);
           exTh/l, eoTh/l = 3-pass projection + bias -> DRAM scratch
  phase B, per l-block of 512 rows:
     S = 3-pass align matmul (PSUM); chunk-max softmax; E (f32r)
     E PE-transposed -> ET;  out = ET.T @ iother tiles, scaled by 1/Z
"""

import numpy as np

import concourse.bass as bass
import concourse.mybir as mybir
import concourse.tile as tile
from concourse import bacc

P = 128          # partitions
L = 2048         # sequence length
D = 1024         # feature dim
NB = 8           # batch / cores
KC = D // P      # 8 contraction chunks for stage-1 matmuls
DG = D // P      # 8 output d-groups of stage 1
NLB = L // 512   # 4 l-blocks of 512
LS = 4           # l-subs of 128 per l-block
MC = L // 512    # 4 m-chunks of 512 for align
M16 = L // P     # 16 m-chunks of 128 for stage 4

F32 = mybir.dt.float32
F32R = mybir.dt.float32r
AX = mybir.AxisListType.X
EXP = mybir.ActivationFunctionType.Exp


def build_program(zero_bias=False):
    """zero_bias=True uses the G = W^T@W factorization:
    ex@eother^T = ix@G@iother^T (+ bias terms that vanish for b=0, up to a
    softmax-invariant per-row shift).  This removes the eother projection
    and all W transposes; G costs half an eother projection."""
    nc = bacc.Bacc("TRN2", target_bir_lowering=False, debug=False)

    ix = nc.dram_tensor("ix", [L, D], F32, kind="ExternalInput").ap()
    iother = nc.dram_tensor("iother", [L, D], F32, kind="ExternalInput").ap()
    W = nc.dram_tensor("W", [D, D], F32, kind="ExternalInput").ap()
    bvec = nc.dram_tensor("b", [D], F32, kind="ExternalInput").ap()
    out = nc.dram_tensor("out", [L, D], F32, kind="ExternalOutput").ap()
    # identity for PE transpose-mode, fed from host (avoids f32r memset)
    ident_in = nc.dram_tensor("ident", [P, P], F32, kind="ExternalInput").ap()

    # staging: projected-transposed activations (hi/lo), phase A -> phase B.
    # One DRAM tensor per 512-block so Tile's per-tensor DRAM dependency
    # tracking lets phase-B reads start as soon as their block is written.
    def scratch(name):
        t = nc.dram_tensor(name, [D, 512], F32R).ap()
        return t.rearrange("(dg p) l -> p dg l", p=P)           # [128, 8, 512]

    exT_h = [scratch(f"exTh_scratch{i}") for i in range(NLB)]
    exT_l = [scratch(f"exTl_scratch{i}") for i in range(NLB)]
    eoT_h = [scratch(f"eoTh_scratch{i}") for i in range(NLB)]
    eoT_l = [scratch(f"eoTl_scratch{i}") for i in range(NLB)]

    with tile.TileContext(nc) as tc:
        with (
            tc.tile_pool(name="const", bufs=1) as const,
            tc.tile_pool(name="exTb", bufs=1) as exTb_pool,
            tc.tile_pool(name="eoTb", bufs=2) as eoTb_pool,
            tc.tile_pool(name="psum_tp", bufs=2, space="PSUM") as psum_tp,
            tc.tile_pool(name="psum_mm", bufs=2, space="PSUM") as psum_mm,
            tc.tile_pool(name="psum_s4", bufs=4, space="PSUM") as psum_s4,
        ):
            identf = const.tile([P, P], F32, name="identf")
            nc.sync.dma_start(out=identf, in_=ident_in)

            # b laid out [p, dg]: btile[p, dg] = b[dg*128 + p]
            btile = const.tile([P, DG], F32)
            nc.sync.dma_start(out=btile, in_=bvec.rearrange("(c p) -> p c", p=P))

            def transpose_128_group(src_row, dst):
                """Transpose four [128,128] fp32 slices through one PSUM bank
                (fp32 transpose-mode is bit-exact); single eviction into dst
                ([128, 4, 128] SBUF AP) -- an f32r dst rounds on write."""
                tp = psum_tp.tile([P, 4 * P], F32, tag="tp")
                for i in range(4):
                    nc.tensor.transpose(
                        tp[:, i * P:(i + 1) * P],
                        src_row[:, i * P:(i + 1) * P],
                        identf,
                    )
                nc.scalar.copy(out=dst, in_=tp.rearrange(
                    "p (four c) -> p four c", four=4))

            def transpose_128_group_hl(src_row, dst_h, dst_l):
                """Like transpose_128_group, but evicts an f32r hi/lo pair:
                hi = rne11(psum) via ACT copy, lo = psum - hi via DVE sub."""
                tp = psum_tp.tile([P, 4 * P], F32, tag="tp")
                for i in range(4):
                    nc.tensor.transpose(
                        tp[:, i * P:(i + 1) * P],
                        src_row[:, i * P:(i + 1) * P],
                        identf,
                    )
                tp4 = tp.rearrange("p (four c) -> p four c", four=4)
                nc.scalar.copy(out=dst_h, in_=tp4)
                nc.vector.tensor_sub(out=dst_l, in0=tp4, in1=dst_h)

            # ---------------- phase A: WTh/WTl + exT/eoT (hi/lo) -> DRAM ----
            with (
                tc.tile_pool(name="wt", bufs=1) as wt_pool,
                tc.tile_pool(name="stage", bufs=3) as stage_pool,
                tc.tile_pool(name="split", bufs=2) as split_pool,
                tc.tile_pool(name="xT", bufs=1) as xT_pool,
                tc.tile_pool(name="ev", bufs=1) as ev_pool,
                tc.tile_pool(name="evt", bufs=2) as evt_pool,
            ):
                # lhsT hi/lo pair for the ix projection:
                #   direct path: WT (transposed W);  G path: G = W^T@W
                #   (symmetric, so its [i-part, j] layout is its own lhsT)
                wth = wt_pool.tile([P, KC, D], F32R)
                wtl = wt_pool.tile([P, KC, D], F32R)

                if zero_bias:
                    # G = W^T @ W, hi/lo 3-pass over W rows; j in two halves
                    # so 8 PSUM groups suffice (uses all three psum pools).
                    for jh in range(2):
                        jsl = slice(jh * 512, (jh + 1) * 512)
                        pss = ([psum_mm.tile([P, 512], F32, tag="mm",
                                             name=f"g{jh}_{i}")
                                for i in range(2)]
                               + [psum_s4.tile([P, 512], F32, tag="s4",
                                               name=f"g{jh}_{i + 2}")
                                  for i in range(4)]
                               + [psum_tp.tile([P, 512], F32, tag="tp",
                                               name=f"g{jh}_{i + 6}")
                                  for i in range(2)])
                        for dc in range(DG):
                            wrow = stage_pool.tile([P, D], F32, tag="stage",
                                                   name=f"gw{jh}_{dc}")
                            nc.sync.dma_start(
                                out=wrow, in_=W[dc * P:(dc + 1) * P, :])
                            whc = split_pool.tile([P, D], F32R, tag="whc",
                                                  name=f"gwh{jh}_{dc}")
                            nc.vector.tensor_copy(out=whc, in_=wrow)
                            wlc = split_pool.tile([P, D], F32R, tag="wlc",
                                                  name=f"gwl{jh}_{dc}")
                            nc.vector.tensor_sub(out=wlc, in0=wrow, in1=whc)
                            for ic in range(DG):
                                isl = slice(ic * P, (ic + 1) * P)
                                for n, (a_t, b_t) in enumerate(
                                        ((whc, whc), (whc, wlc), (wlc, whc))):
                                    nc.tensor.matmul(
                                        pss[ic], a_t[:, isl], b_t[:, jsl],
                                        start=(dc == 0 and n == 0),
                                        stop=(dc == DG - 1 and n == 2),
                                    )
                        for ic in range(DG):
                            tmp = evt_pool.tile([P, 512], F32, tag="evt",
                                                name=f"gt{jh}_{ic}")
                            nc.vector.tensor_copy(out=tmp, in_=pss[ic])
                            nc.scalar.copy(out=wth[:, ic, jsl], in_=tmp)
                            nc.vector.tensor_sub(out=wtl[:, ic, jsl],
                                                 in0=tmp, in1=wth[:, ic, jsl])
                else:
                    for dc in range(DG):
                        wrow = stage_pool.tile([P, D], F32, tag="stage",
                                               name=f"wrow{dc}")
                        nc.sync.dma_start(out=wrow,
                                          in_=W[dc * P:(dc + 1) * P, :])
                        for q in range(2):
                            transpose_128_group_hl(
                                wrow[:, q * 4 * P:(q + 1) * 4 * P],
                                wth[:, q * 4:(q + 1) * 4, dc * P:(dc + 1) * P],
                                wtl[:, q * 4:(q + 1) * 4, dc * P:(dc + 1) * P])

                def stage1(src_dram, dst_h, dst_l, pfx, project=True):
                    """project=True: dst = lhsT_pair @ src_blk^T + b (3-pass
                    hi/lo), stored hi/lo.  project=False: dst = src_blk^T
                    only (split+transpose, no matmul)."""
                    for blk in range(NLB):
                        xh = xT_pool.tile([P, KC, 512], F32R, tag="xh",
                                          name=f"{pfx}xh{blk}")
                        xl = xT_pool.tile([P, KC, 512], F32R, tag="xl",
                                          name=f"{pfx}xl{blk}")
                        for s in range(4):
                            row = stage_pool.tile([P, D], F32, tag="stage",
                                                  name=f"{pfx}row{blk}_{s}")
                            r0 = (blk * 4 + s) * P
                            nc.sync.dma_start(out=row,
                                              in_=src_dram[r0:r0 + P, :])
                            ssl = slice(s * P, (s + 1) * P)
                            for q in range(2):
                                transpose_128_group_hl(
                                    row[:, q * 4 * P:(q + 1) * 4 * P],
                                    xh[:, q * 4:(q + 1) * 4, ssl],
                                    xl[:, q * 4:(q + 1) * 4, ssl])
                        if not project:
                            nc.sync.dma_start(out=dst_h[blk], in_=xh)
                            nc.sync.dma_start(out=dst_l[blk], in_=xl)
                            continue
                        for dg in range(DG):
                            if dg % 2 == 0:
                                evh = ev_pool.tile(
                                    [P, 2, 512], F32R, tag="evh",
                                    name=f"{pfx}evh{blk}_{dg}")
                                evl = ev_pool.tile(
                                    [P, 2, 512], F32R, tag="evl",
                                    name=f"{pfx}evl{blk}_{dg}")
                            ps = psum_mm.tile([P, 512], F32, tag="mm",
                                              name=f"{pfx}ps{blk}_{dg}")
                            n = 0
                            for wt_t, x_t in ((wth, xh), (wth, xl), (wtl, xh)):
                                for kc in range(KC):
                                    nc.tensor.matmul(
                                        ps,
                                        wt_t[:, kc, dg * P:(dg + 1) * P],
                                        x_t[:, kc, :],
                                        start=(n == 0), stop=(n == 3 * KC - 1),
                                    )
                                    n += 1
                            tmp = evt_pool.tile([P, 512], F32, tag="evt",
                                                name=f"{pfx}tmp{blk}_{dg}")
                            nc.vector.tensor_scalar_add(
                                out=tmp, in0=ps, scalar1=btile[:, dg:dg + 1])
                            nc.vector.tensor_copy(out=evh[:, dg % 2, :],
                                                  in_=tmp)
                            nc.vector.tensor_sub(out=evl[:, dg % 2, :],
                                                 in0=tmp,
                                                 in1=evh[:, dg % 2, :])
                            if dg % 2 == 1:
                                dsl = slice(dg - 1, dg + 1)
                                nc.sync.dma_start(
                                    out=dst_h[blk][:, dsl, :], in_=evh)
                                nc.sync.dma_start(
                                    out=dst_l[blk][:, dsl, :], in_=evl)

                if zero_bias:
                    stage1(iother, eoT_h, eoT_l, "eo", project=False)
                    stage1(ix, exT_h, exT_l, "ex")
                else:
                    stage1(ix, exT_h, exT_l, "ex")
                    stage1(iother, eoT_h, eoT_l, "eo")

            # ---------------- phase B: align + softmax + output -------------
            with (
                tc.tile_pool(name="epool", bufs=4) as e_pool,
                tc.tile_pool(name="etpool", bufs=4) as et_pool,
                tc.tile_pool(name="s4rhs", bufs=6) as s4rhs_pool,
                tc.tile_pool(name="outp", bufs=6) as out_pool,
                tc.tile_pool(name="small", bufs=10) as small_pool,
            ):
                for lb in range(NLB):
                    exbh = exTb_pool.tile([P, DG, 512], F32R, tag="exbh",
                                          name=f"exbh{lb}")
                    exbl = exTb_pool.tile([P, DG, 512], F32R, tag="exbl",
                                          name=f"exbl{lb}")
                    # SWDGE queue: lets these overtake phase-A writes still
                    # pending in the sync-engine HWDGE FIFO
                    nc.gpsimd.dma_start(out=exbh, in_=exT_h[lb])
                    nc.gpsimd.dma_start(out=exbl, in_=exT_l[lb])

                    NMC = 2 * MC      # 8 chunks of 256
                    Es = [e_pool.tile([P, L], F32, tag="E",
                                      name=f"E{lb}_{i}") for i in range(LS)]
                    # per-chunk maxes, one [128, NMC] tile per l-sub
                    mcms = [small_pool.tile([P, NMC], F32, tag="mcm",
                                            name=f"mcm{lb}_{i}")
                            for i in range(LS)]
                    for mc in range(NMC):
                        msl = slice(mc * 256, (mc + 1) * 256)
                        blk_i, half = mc // 2, mc % 2
                        hsl = slice(half * 256, (half + 1) * 256)
                        eobh = eoTb_pool.tile([P, DG, 256], F32R, tag="eobh",
                                              name=f"eobh{lb}_{mc}")
                        eobl = eoTb_pool.tile([P, DG, 256], F32R, tag="eobl",
                                              name=f"eobl{lb}_{mc}")
                        nc.gpsimd.dma_start(out=eobh,
                                            in_=eoT_h[blk_i][:, :, hsl])
                        nc.gpsimd.dma_start(out=eobl,
                                            in_=eoT_l[blk_i][:, :, hsl])
                        for ls in range(LS):
                            ps = psum_mm.tile([P, 256], F32, tag="mm",
                                              name=f"al{lb}_{mc}_{ls}")
                            n = 0
                            for x_t, eo_t in ((exbh, eobh), (exbh, eobl),
                                              (exbl, eobh)):
                                for dc in range(DG):
                                    nc.tensor.matmul(
                                        ps,
                                        x_t[:, dc, ls * P:(ls + 1) * P],
                                        eo_t[:, dc, :],
                                        start=(n == 0), stop=(n == 3 * DG - 1),
                                    )
                                    n += 1
                            # chunk-relative storage: E_c = S_c - max(S_c)
                            nc.vector.reduce_max(
                                mcms[ls][:, mc:mc + 1], ps, axis=AX)
                            nc.vector.tensor_scalar_sub(
                                out=Es[ls][:, msl], in0=ps,
                                scalar1=mcms[ls][:, mc:mc + 1])

                    ets = []
                    rzs = []
                    for ls in range(LS):
                        E = Es[ls]
                        # row max M = max_c mcm; biases = mcm - M
                        negM = small_pool.tile([P, 1], F32, tag="negM",
                                               name=f"nm{lb}_{ls}")
                        nc.vector.reduce_max(negM, mcms[ls], axis=AX,
                                             negate=True)
                        biases = small_pool.tile([P, NMC], F32, tag="biases",
                                                 name=f"bi{lb}_{ls}")
                        nc.vector.tensor_scalar_add(out=biases, in0=mcms[ls],
                                                    scalar1=negM)
                        zc = small_pool.tile([P, NMC], F32, tag="zc",
                                             name=f"zc{lb}_{ls}")
                        for c in range(NMC):
                            csl = slice(c * 256, (c + 1) * 256)
                            nc.scalar.activation(
                                out=E[:, csl], in_=E[:, csl], func=EXP,
                                bias=biases[:, c:c + 1], scale=1.0,
                                accum_out=zc[:, c:c + 1])
                        zsum = small_pool.tile([P, 1], F32, tag="zsum",
                                               name=f"zs{lb}_{ls}")
                        nc.vector.reduce_sum(zsum, zc, axis=AX)
                        rz = small_pool.tile([P, 1], F32, tag="rz",
                                             name=f"rz{lb}_{ls}")
                        nc.vector.reciprocal(rz, zsum)
                        rzs.append(rz)
                        # ET[p, m16, l] = E[l, m16*128 + p]
                        ET = et_pool.tile([P, M16, P], F32R, tag="ET",
                                          name=f"ET{lb}_{ls}")
                        for q in range(4):
                            transpose_128_group(
                                E[:, q * 4 * P:(q + 1) * 4 * P],
                                ET[:, q * 4:(q + 1) * 4, :])
                        ets.append(ET)

                    # stage 4: out rows = (E @ iother) * rz
                    for dg in range(2):
                        pss = [psum_s4.tile([P, 512], F32, tag="s4",
                                            name=f"s4_{lb}_{dg}_{i}")
                               for i in range(LS)]
                        for m16 in range(M16):
                            rhs = s4rhs_pool.tile([P, 512], F32R, tag="s4rhs",
                                                  name=f"rhs{lb}_{dg}_{m16}")
                            nc.sync.dma_start(
                                out=rhs,
                                in_=iother[m16 * P:(m16 + 1) * P,
                                           dg * 512:(dg + 1) * 512].bitcast(F32R))
                            for ls in range(LS):
                                nc.tensor.matmul(
                                    pss[ls],
                                    ets[ls][:, m16, :],
                                    rhs,
                                    start=(m16 == 0), stop=(m16 == M16 - 1),
                                )
                        for ls in range(LS):
                            ot = out_pool.tile([P, 512], F32, tag="ot",
                                               name=f"ot{lb}_{dg}_{ls}")
                            nc.vector.tensor_scalar_mul(
                                out=ot, in0=pss[ls], scalar1=rzs[ls])
                            r0 = lb * 512 + ls * P
                            nc.sync.dma_start(
                                out=out[r0:r0 + P, dg * 512:(dg + 1) * 512],
                                in_=ot)

    nc.compile()
    return nc


_NC_CACHE = {}


def _get_nc(zero_bias):
    if zero_bias not in _NC_CACHE:
        _NC_CACHE[zero_bias] = build_program(zero_bias)
    return _NC_CACHE[zero_bias]


def kernel(ix, iother, W, b):
    """Full-input entry point: shards batch across 8 NeuronCores."""
    from concourse.bass_utils import run_bass_kernel_spmd

    ix = np.ascontiguousarray(np.asarray(ix, dtype=np.float32))
    iother = np.ascontiguousarray(np.asarray(iother, dtype=np.float32))
    W = np.ascontiguousarray(np.asarray(W, dtype=np.float32))
    b = np.ascontiguousarray(np.asarray(b, dtype=np.float32))

    nc = _get_nc(zero_bias=bool(np.all(b == 0.0)))
    core_ids = list(range(NB))
    ident = np.eye(P, dtype=np.float32)
    in_maps = [
        {"ix": ix[i], "iother": iother[i], "W": W, "b": b, "ident": ident}
        for i in range(NB)
    ]
    res = run_bass_kernel_spmd(nc, in_maps, core_ids)
    outs = [res.results[i]["out"] for i in range(NB)]
    return np.stack(outs, axis=0).astype(np.float32)
